# revision 18
# baseline (speedup 1.0000x reference)
"""GAT (3-layer, 8-head) forward on 8 Trainium2 NeuronCores.

Architecture (v2 — instruction-count-minimized):
  - Nodes partitioned across 8 cores (graph parallel); per-core permutation
    sorts nodes by in-degree so adjacent 128-node tiles have similar max
    degree K.
  - Tiles are grouped into blocks of m tiles padded to a common slot count
    Kb; each block's whole neighbor gather is ONE batched indirect DMA
    (offset AP [128, m*Kb], one descriptor per edge slot).
  - Per layer: node phase projects features + attention dots with one
    matmul per 128-node tile against combined [WA|W|WD] (grouped 3-4 tiles
    per PSUM bank); the [als|h] table is AllGathered so every core can
    gather any source row.
  - Edge phase per block: softmax (no max-subtraction; logits bounded),
    alpha-weighting in place, then slot-axis segment reduction via ONE
    strided-view vector tensor_reduce (slot axis made innermost by AP
    permutation) — no per-slot matmuls.
  - Matmul inputs (x, h, weights) are bf16 (f32 PSUM accumulate); tables
    and softmax math stay f32; final output is f16 (cast to f32 on host).
  - Padding slots gather a dummy row (als=-100 -> exp ~ 0, h=0).
"""
import sys

sys.path.insert(0, "/opt/trn_rl_repo")

import numpy as np
import ml_dtypes

import jax
import jax.numpy as jnp
from jax.sharding import Mesh, PartitionSpec, NamedSharding
from jax.experimental.shard_map import shard_map

import concourse.bacc as bacc
import concourse.bass2jax as b2j
import concourse.tile as tile
from concourse import mybir
from concourse.bass import IndirectOffsetOnAxis, ds
from concourse.masks import make_identity

# Warm the one-time cffi/pycparser ISA tables at import (~0.9 s) so the
# first Bacc build inside kernel() doesn't pay for it.
try:
    bacc.Bacc("TRN2", target_bir_lowering=False, debug=False,
              num_devices=1).isa
except Exception:
    pass

AF = mybir.ActivationFunctionType
ALU = mybir.AluOpType
AX = mybir.AxisListType

P = 128
NCORES = 8
LRELU = 0.2
LN_EPS = 1e-5

# problem dims (hardcoded per contract)
N_FULL = 100000
D_IN = 128
D_OUT = 64

SLOTS = 96     # max padded slots per block (m * Kb)
MBLK = 12      # max tiles per edge block
GB = 16        # gather-loop batch: columns fetched per For_i iteration

F16 = np.float16


# --------------------------------------------------------------------------
# host-side graph layout
# --------------------------------------------------------------------------

def prepare_layout(edge_index: np.ndarray, n: int):
    npc = n // NCORES
    nloc = ((npc + 1 + P - 1) // P) * P       # >=1 pad row per core
    nt = nloc // P
    nrows = NCORES * nloc

    loops = np.arange(n, dtype=np.int32)
    src = np.concatenate([loops, edge_index[0].astype(np.int32)])
    dst = np.concatenate([loops, edge_index[1].astype(np.int32)])

    deg = np.bincount(dst, minlength=n).astype(np.int32)  # incl self-loop

    dg = deg.reshape(NCORES, npc)
    order = np.argsort(dg, axis=1, kind="stable")                 # [8, npc]
    olds_sorted = order + (np.arange(NCORES) * npc)[:, None]      # old ids
    new_id = np.empty(n, np.int32)
    new_mat = np.arange(npc, dtype=np.int32)[None, :] + \
        (np.arange(NCORES, dtype=np.int32) * nloc)[:, None]
    new_id[olds_sorted.ravel()] = new_mat.ravel()

    nsrc = new_id[src]
    ndst = new_id[dst]

    degn = np.zeros(nrows, np.int32)
    degn[new_id] = deg
    K = degn.reshape(NCORES, nt, P).max(axis=(0, 2))
    K = np.maximum(K, 1).astype(np.int64)
    slots = max(SLOTS, int(K.max()))

    # greedy blocks of adjacent tiles padded to the block max degree
    blocks = []            # (t0, m, Kb, boff)
    colof = np.zeros(nt, np.int32)
    boff = 0
    t0 = 0
    while t0 < nt:
        m = 1
        Kb = int(K[t0])
        while (t0 + m < nt and m < MBLK
               and (m + 1) * max(Kb, int(K[t0 + m])) <= slots):
            Kb = max(Kb, int(K[t0 + m]))
            m += 1
        for j in range(m):
            colof[t0 + j] = boff + j * Kb
        blocks.append((t0, m, Kb, boff))
        boff += m * Kb
        t0 += m
    SUMK = ((boff + GB - 1) // GB) * GB   # pad so the gather loop tiles evenly

    idx = np.empty((NCORES, P, SUMK), dtype=np.int32)
    dummy = (np.arange(NCORES) * nloc + nloc - 1).astype(np.int32)
    idx[:] = dummy[:, None, None]

    order2 = np.argsort(ndst, kind="stable")
    s2 = nsrc[order2]
    d2 = ndst[order2]
    run_start = np.searchsorted(d2, np.arange(nrows, dtype=np.int32)).astype(np.int32)
    slot = np.arange(len(d2), dtype=np.int32) - run_start[d2]
    c_arr = d2 // nloc
    rank = d2 % nloc
    cols = colof[rank // P] + slot
    idx[c_arr, rank % P, cols] = s2

    return {
        "n": n, "npc": npc, "nloc": nloc, "nt": nt, "nrows": nrows,
        "olds_sorted": olds_sorted, "blocks": blocks, "SUMK": SUMK,
        "idx": idx, "K": K,
    }


# --------------------------------------------------------------------------
# device program
# --------------------------------------------------------------------------

class LayerSpec:
    def __init__(self, heads, ch, last, use_bias, use_gamma, use_beta):
        self.heads = heads
        self.ch = ch
        self.dh = heads * ch
        self.row = heads + self.dh         # [als(H) | h(dh)]
        self.ncols = self.row + heads      # + ald(H)
        self.last = last
        self.use_bias = use_bias
        self.use_gamma = use_gamma
        self.use_beta = use_beta


def build_nc(layout, specs):
    nloc, nt, nrows = layout["nloc"], layout["nt"], layout["nrows"]
    blocks, SUMK = layout["blocks"], layout["SUMK"]
    f32 = mybir.dt.float32
    f16 = mybir.dt.float16

    nc = bacc.Bacc("TRN2", target_bir_lowering=False, debug=False,
                   num_devices=NCORES)

    # ---- external I/O ----
    xT_d = nc.dram_tensor("xT", [P, nloc], f16, kind="ExternalInput")
    idx_d = nc.dram_tensor("idx", [P, SUMK], mybir.dt.int32, kind="ExternalInput")
    wall_d = [nc.dram_tensor(f"wall{i}", [P, s.ncols], f16, kind="ExternalInput")
              for i, s in enumerate(specs)]
    auxw = 32 + (3 * P * len(specs)
                 if any(s.use_bias or s.use_gamma or s.use_beta for s in specs)
                 else 0)
    aux_d = nc.dram_tensor("aux", [P, auxw], f32, kind="ExternalInput")
    # aux cols: [0:8]=-100 dummy als, [8]=LN eps, 32+li*384: [bias|gamma|beta]
    out_d = nc.dram_tensor("out", [nloc, specs[-1].dh], f16, kind="ExternalOutput")

    with tile.TileContext(nc) as tc:
        import contextlib
        ctx = contextlib.ExitStack()
        with ctx:
            cpool = ctx.enter_context(tc.tile_pool(name="const", bufs=1))
            dram = ctx.enter_context(tc.tile_pool(name="dram", bufs=1, space="DRAM"))
            npsum = ctx.enter_context(tc.tile_pool(name="npsum", bufs=2, space="PSUM"))
            tpsum = ctx.enter_context(tc.tile_pool(name="tpsum", bufs=2, space="PSUM"))
            gpool = ctx.enter_context(tc.tile_pool(name="gpool", bufs=2))
            work = ctx.enter_context(tc.tile_pool(name="work", bufs=2))
            spool = ctx.enter_context(tc.tile_pool(name="small", bufs=2))

            # ---- persistent SBUF ----
            hin = cpool.tile([P, nloc], f16)
            nc.sync.dma_start(hin[:], xT_d[:])
            idx_sb = cpool.tile([P, SUMK], mybir.dt.int32)
            nc.sync.dma_start(idx_sb[:], idx_d[:])
            aux = cpool.tile([P, auxw], f32)
            nc.sync.dma_start(aux[:], aux_d[:])
            identb = cpool.tile([P, P], f16)
            make_identity(nc, identb[:])
            ald_sb = cpool.tile([P, nt * 8], f32)
            ald2_sb = cpool.tile([P, nt], f32)

            walls = []
            for i, s in enumerate(specs):
                w = cpool.tile([P, s.ncols], f16, name=f"wall{i}_sb")
                nc.sync.dma_start(w[:], wall_d[i][:])
                walls.append(w)

            # per-layer DRAM tables
            tls = [dram.tile([nloc, s.row], f32, name=f"tl{i}")
                   for i, s in enumerate(specs)]
            tfs = [dram.tile([nrows, s.row], f32, name=f"tf{i}", addr_space="Shared")
                   for i, s in enumerate(specs)]

            # gather staging (data indirection: the indirect DMA's offset AP
            # stays static; a per-iteration copy feeds it fresh indices)
            gidx = cpool.tile([P, GB], mybir.dt.int32)
            grows = cpool.tile([P, GB, 136], f32)

            for li, s in enumerate(specs):
                wall = walls[li]
                H, ch, dh, row = s.heads, s.ch, s.dh, s.row
                tl, tf = tls[li], tfs[li]
                ald = ald_sb if H == 8 else ald2_sb

                # ---------- node phase (groups of gsz tiles per PSUM bank) ----
                gsz = 512 // s.ncols
                for g0 in range(0, nt, gsz):
                    m = min(gsz, nt - g0)
                    pn = npsum.tile([P, gsz, s.ncols], f32, tag="pn")
                    for j in range(m):
                        t = g0 + j
                        nc.tensor.matmul(out=pn[:, j, :],
                                         lhsT=hin[:, t * P:(t + 1) * P],
                                         rhs=wall[:], start=True, stop=True)
                    stage = work.tile([P, gsz, row], f32, tag="stage")
                    nc.scalar.copy(stage[:, :m, :], pn[:, :m, 0:row])
                    nc.scalar.copy(
                        ald[:, g0 * H:(g0 + m) * H].rearrange(
                            "p (m h) -> p m h", m=m),
                        pn[:, :m, row:row + H])
                    nc.sync.dma_start(
                        tl[g0 * P:(g0 + m) * P, :].rearrange(
                            "(j p) r -> p j r", p=P),
                        stage[:, :m, :])

                # dummy row: overwrite als cols of last row with -100
                nc.sync.dma_start(tl[nloc - 1:nloc, 0:H],
                                  aux[0:1, 0:H])

                # ---------- allgather ----------
                # drain in-flight SWDGE DMAs: a collective triggered with
                # indirect-DMA descriptors in flight crashes the exec unit
                nc.gpsimd.dma_reset()
                nc.gpsimd.collective_compute(
                    "AllGather", ALU.bypass,
                    ins=[tl[:]], outs=[tf[:]],
                    replica_groups=[list(range(NCORES))],
                )

                # ---------- gather loop: stream all edge rows to DRAM ----
                gedge = dram.tile([P, SUMK, row], f32, tag="gedge",
                                  name=f"gedge{li}")
                with tc.For_i(0, SUMK, GB) as it:
                    nc.vector.tensor_copy(gidx[:], idx_sb[:, ds(it, GB)])
                    for b_ in range(GB):
                        nc.gpsimd.indirect_dma_start(
                            out=grows[:, b_, 0:row], out_offset=None,
                            in_=tf[:],
                            in_offset=IndirectOffsetOnAxis(
                                ap=gidx[:, b_:b_ + 1], axis=0),
                        )
                    nc.sync.dma_start(gedge[:, ds(it, GB), :],
                                      grows[:, :, 0:row])

                # ---------- edge phase (per block) ----------
                for (t0, m, Kb, boff) in blocks:
                    S = m * Kb
                    g = gpool.tile([P, S, row], f32, tag="g")
                    nc.sync.dma_start(g[:], gedge[:, boff:boff + S, :])
                    # logits l = als + ald  ([P, m, Kb, H] views)
                    lsb = work.tile([P, S, H], f32, tag="lsb")
                    nc.vector.tensor_tensor(
                        lsb[:].rearrange("p (m k) h -> p m k h", m=m),
                        g[:, :, 0:H].rearrange("p (m k) h -> p m k h", m=m),
                        ald[:, t0 * H:(t0 + m) * H].rearrange(
                            "p (m h) -> p m h", m=m)[:, :, None, :]
                        .to_broadcast([P, m, Kb, H]),
                        ALU.add)
                    # leaky relu: (l * 0.2) max l ; then ee = exp(l)
                    nc.vector.scalar_tensor_tensor(
                        lsb[:], lsb[:], LRELU, lsb[:], op0=ALU.mult, op1=ALU.max)
                    nc.scalar.activation(lsb[:], lsb[:], AF.Exp)
                    # msg h *= ee (per head)
                    gh = g[:, :, H:row].rearrange("p s (h c) -> p s h c", h=H)
                    nc.vector.tensor_tensor(
                        gh, gh,
                        lsb[:, :, :, None].to_broadcast([P, S, H, ch]),
                        ALU.mult)
                    # denominators: reduce ee over slot axis (innermost view)
                    den = spool.tile([P, m, H], f32, tag="den")
                    nc.vector.tensor_reduce(
                        den[:],
                        lsb[:].rearrange("p (m k) h -> p m h k", m=m),
                        axis=AX.X, op=ALU.add)
                    # messages: reduce weighted h over slot axis
                    msg = work.tile([P, m, dh], f32, tag="msg")
                    nc.vector.tensor_reduce(
                        msg[:],
                        g[:, :, H:row].rearrange("p (m k) r -> p m r k", m=m),
                        axis=AX.X, op=ALU.add)
                    # normalize by denominator
                    rec = spool.tile([P, m, H], f32, tag="rec")
                    nc.vector.reciprocal(rec[:], den[:])
                    msg4 = msg[:].rearrange("p m (h c) -> p m h c", h=H)
                    nc.vector.tensor_tensor(
                        msg4, msg4,
                        rec[:, :, :, None].to_broadcast([P, m, H, ch]),
                        ALU.mult)
                    if s.use_bias:
                        nc.vector.tensor_tensor(
                            msg[:], msg[:],
                            aux[:, None, 32 + li * 3 * P:32 + li * 3 * P + dh]
                            .to_broadcast([P, m, dh]),
                            ALU.add)

                    if not s.last:
                        # ---- layer norm + relu (per block, vector ops) ----
                        s1 = spool.tile([P, m], f32, tag="s1")
                        nc.vector.tensor_reduce(s1[:], msg[:], axis=AX.X,
                                                op=ALU.add)
                        sq = work.tile([P, m, dh], f32, tag="sq")
                        nc.scalar.activation(sq[:], msg[:], AF.Square)
                        s2 = spool.tile([P, m], f32, tag="s2")
                        nc.vector.tensor_reduce(s2[:], sq[:], axis=AX.X,
                                                op=ALU.add)
                        mu = spool.tile([P, m], f32, tag="mu")
                        nc.vector.tensor_scalar_mul(mu[:], s1[:], 1.0 / dh)
                        ex2 = spool.tile([P, m], f32, tag="ex2")
                        nc.vector.tensor_scalar_mul(ex2[:], s2[:], 1.0 / dh)
                        mu2 = spool.tile([P, m], f32, tag="mu2")
                        nc.vector.tensor_tensor(mu2[:], mu[:], mu[:], ALU.mult)
                        var = spool.tile([P, m], f32, tag="var")
                        nc.vector.tensor_tensor(var[:], ex2[:], mu2[:],
                                                ALU.subtract)
                        sd = spool.tile([P, m], f32, tag="sd")
                        nc.scalar.activation(sd[:], var[:], AF.Sqrt,
                                             bias=aux[:, 8:9])
                        rstd = spool.tile([P, m], f32, tag="rstd")
                        nc.vector.reciprocal(rstd[:], sd[:])
                        # xn = (msg - mu) * rstd  (reuse sq buffer)
                        nc.vector.tensor_tensor(
                            sq[:], msg[:],
                            mu[:, :, None].to_broadcast([P, m, dh]),
                            ALU.subtract)
                        nc.vector.tensor_tensor(
                            sq[:], sq[:],
                            rstd[:, :, None].to_broadcast([P, m, dh]),
                            ALU.mult)
                        if s.use_gamma:
                            nc.vector.tensor_tensor(
                                sq[:], sq[:],
                                aux[:, None, 32 + li * 3 * P + P:
                                    32 + li * 3 * P + P + dh]
                                .to_broadcast([P, m, dh]), ALU.mult)
                        if s.use_beta:
                            nc.vector.tensor_tensor(
                                sq[:], sq[:],
                                aux[:, None, 32 + li * 3 * P + 2 * P:
                                    32 + li * 3 * P + 2 * P + dh]
                                .to_broadcast([P, m, dh]), ALU.add)
                        hn = work.tile([P, m, dh], f16, tag="hn")
                        nc.vector.tensor_scalar_max(hn[:], sq[:], 0.0)
                        # transpose each tile back into hin (feature-major)
                        for j in range(m):
                            pt = tpsum.tile([P, P], f16, tag="pt")
                            nc.tensor.transpose(pt[:], hn[:, j, :], identb[:])
                            nc.scalar.copy(
                                hin[:, (t0 + j) * P:(t0 + j + 1) * P], pt[:])
                    else:
                        # ---- log_softmax + output DMA ----
                        mxn = spool.tile([P, m], f32, tag="mxn")
                        nc.vector.tensor_reduce(mxn[:], msg[:], axis=AX.X,
                                                op=ALU.max, negate=True)
                        tsb = work.tile([P, m, dh], f32, tag="tsb")
                        nc.vector.tensor_tensor(
                            tsb[:], msg[:],
                            mxn[:, :, None].to_broadcast([P, m, dh]),
                            ALU.add)
                        nc.scalar.activation(msg[:], tsb[:], AF.Exp)
                        ssum = spool.tile([P, m], f32, tag="ssum")
                        nc.vector.tensor_reduce(ssum[:], msg[:], axis=AX.X,
                                                op=ALU.add)
                        lns = spool.tile([P, m], f32, tag="lns")
                        nc.scalar.activation(lns[:], ssum[:], AF.Ln)
                        of = work.tile([P, m, dh], f16, tag="of")
                        nc.vector.tensor_tensor(
                            of[:], tsb[:],
                            lns[:, :, None].to_broadcast([P, m, dh]),
                            ALU.subtract)
                        nc.sync.dma_start(
                            out_d[t0 * P:(t0 + m) * P, :].rearrange(
                                "(j p) c -> p j c", p=P),
                            of[:])

    nc.compile()
    return nc


# --------------------------------------------------------------------------
# host wrapper
# --------------------------------------------------------------------------

def _block_diag_a(a, heads, ch):
    """[heads*ch, heads]: col h nonzero only on head h's channels."""
    out = np.zeros((heads * ch, heads), dtype=np.float32)
    for h in range(heads):
        out[h * ch:(h + 1) * ch, h] = a[h]
    return out


def _device_mesh():
    devices = jax.devices()[:NCORES]
    mesh = Mesh(np.asarray(devices), ("core",))
    return mesh, NamedSharding(mesh, PartitionSpec("core"))


def _run_pjrt(nc, mesh, sh, dev_in):
    """Execute the prebuilt Bass module via PJRT against inputs that are
    already resident on the devices (adapted from bass2jax.run_bass_via_pjrt,
    minus host-side zero-output transfers)."""
    b2j.install_neuronx_cc_hook()
    partition_name = nc.partition_id_tensor.name if nc.partition_id_tensor else None
    in_names, out_names, out_avals = [], [], []
    for alloc in nc.m.functions[0].allocations:
        if not isinstance(alloc, mybir.MemoryLocationSet):
            continue
        name = alloc.memorylocations[0].name
        if alloc.kind == "ExternalInput":
            if name != partition_name:
                in_names.append(name)
        elif alloc.kind == "ExternalOutput":
            out_names.append(name)
            out_avals.append(jax.core.ShapedArray(
                tuple(alloc.tensor_shape), mybir.dt.np(alloc.dtype)))
    n_params = len(in_names)
    n_outs = len(out_avals)
    all_names = list(in_names) + out_names
    if partition_name is not None:
        all_names.append(partition_name)
    donate = tuple(range(n_params, n_params + n_outs))

    def _body(*args):
        operands = list(args)
        if partition_name is not None:
            operands.append(b2j.partition_id_tensor())
        outs = b2j._bass_exec_p.bind(
            *operands, out_avals=tuple(out_avals), in_names=tuple(all_names),
            out_names=tuple(out_names), lowering_input_output_aliases=(),
            sim_require_finite=True, sim_require_nnan=True, nc=nc)
        return tuple(outs)

    in_specs = (PartitionSpec("core"),) * (n_params + n_outs)
    out_specs = (PartitionSpec("core"),) * n_outs
    fn = jax.jit(shard_map(_body, mesh=mesh, in_specs=in_specs,
                           out_specs=out_specs, check_rep=False),
                 donate_argnums=donate, keep_unused=True)
    # donated output buffers were pre-transferred (see run_gat "__zero_<name>")
    zeros = [dev_in[f"__zero_{nm}"] for nm in out_names]
    args = [dev_in[nm] for nm in in_names] + zeros
    outs = fn(*args)
    return {name: np.asarray(outs[i]).reshape(NCORES, *out_avals[i].shape)
            for i, name in enumerate(out_names)}


def run_gat(inputs, n=N_FULL):
    x = np.asarray(inputs["x"], dtype=np.float32)
    edge_index = np.asarray(inputs["edge_index"])
    lay = prepare_layout(edge_index, n)
    nloc, npc = lay["nloc"], lay["npc"]

    W = [np.asarray(inputs[f"W{i}"], dtype=np.float32) for i in range(3)]
    a_s = [np.asarray(inputs[f"as{i}"], dtype=np.float32) for i in range(3)]
    a_d = [np.asarray(inputs[f"ad{i}"], dtype=np.float32) for i in range(3)]
    b = [np.asarray(inputs[f"b{i}"], dtype=np.float32) for i in range(3)]
    ln_g = [np.asarray(inputs["ln1_g"], np.float32),
            np.asarray(inputs["ln2_g"], np.float32)]
    ln_b = [np.asarray(inputs["ln1_b"], np.float32),
            np.asarray(inputs["ln2_b"], np.float32)]

    hc = [(8, 16), (8, 16), (1, 64)]
    specs = []
    for i, (heads, ch) in enumerate(hc):
        use_bias = bool(np.any(b[i] != 0.0))
        use_g = i < 2 and bool(np.any(ln_g[i] != 1.0))
        use_b = i < 2 and bool(np.any(ln_b[i] != 0.0))
        specs.append(LayerSpec(heads, ch, i == 2, use_bias, use_g, use_b))

    # per-layer combined weights [WA(H) | W(dh) | WD(H)], bf16
    wall_np = []
    for i, s in enumerate(specs):
        din = W[i].shape[0]
        bd_s = _block_diag_a(a_s[i].reshape(s.heads, s.ch), s.heads, s.ch)
        bd_d = _block_diag_a(a_d[i].reshape(s.heads, s.ch), s.heads, s.ch)
        m = np.zeros((P, s.ncols), dtype=np.float32)
        m[:din, 0:s.heads] = W[i] @ bd_s
        m[:din, s.heads:s.heads + s.dh] = W[i]
        m[:din, s.heads + s.dh:] = W[i] @ bd_d
        wall_np.append(m.astype(F16))

    wide = any(s.use_bias or s.use_gamma or s.use_beta for s in specs)
    auxw = 32 + (3 * P * len(specs) if wide else 0)
    aux_np = np.zeros((P, auxw), dtype=np.float32)
    aux_np[:, 0:8] = -100.0
    aux_np[:, 8] = LN_EPS
    if wide:
        for i, s in enumerate(specs):
            aux_np[:, 32 + i * 3 * P:32 + i * 3 * P + s.dh] = b[i][None, :]
            if i < 2:
                aux_np[:, 32 + i * 3 * P + P:32 + i * 3 * P + P + s.dh] = \
                    ln_g[i][None, :]
                aux_np[:, 32 + i * 3 * P + 2 * P:32 + i * 3 * P + 2 * P + s.dh] = \
                    ln_b[i][None, :]

    # per-core transposed bf16 features
    xg = x[lay["olds_sorted"]]          # [8, npc, 128]
    xT_all = np.zeros((NCORES, P, nloc), dtype=F16)
    xT_all[:, :, :npc] = xg.transpose(0, 2, 1)

    # kick off host->device transfers NOW; they stream over the (slow) axon
    # tunnel while we trace + compile the kernel below.
    mesh, sh = _device_mesh()
    rep = lambda a: np.broadcast_to(a, (NCORES,) + a.shape).reshape(
        NCORES * a.shape[0], *a.shape[1:])
    concat = {
        "xT": xT_all.reshape(NCORES * P, nloc),
        "idx": lay["idx"].reshape(NCORES * P, lay["SUMK"]),
        "aux": rep(aux_np),
    }
    for i in range(3):
        concat[f"wall{i}"] = rep(wall_np[i])
    concat["__zero_out"] = np.zeros((NCORES * nloc, specs[-1].dh),
                                    dtype=np.float16)
    names = list(concat.keys())
    put = jax.device_put([concat[k] for k in names], sh)
    dev_in = dict(zip(names, put))

    nc = build_nc(lay, specs)
    if nc.dbg_addr is not None:
        dev_in[nc.dbg_addr.name] = jax.device_put(
            np.zeros((NCORES, 2), np.uint32), sh)

    res = _run_pjrt(nc, mesh, sh, dev_in)

    out = res["out"][:, :npc, :].astype(np.float32)   # [8, npc, 64]
    full = np.empty((n, specs[-1].dh), dtype=np.float32)
    full[lay["olds_sorted"].ravel()] = out.reshape(NCORES * npc, -1)
    return full


def kernel(**inputs) -> np.ndarray:
    return run_gat(inputs, n=N_FULL)


# revision 20
# speedup vs baseline: 1.2140x; 1.2140x over previous
"""GAT (3-layer, 8-head) forward on 8 Trainium2 NeuronCores.

Architecture (v2 — instruction-count-minimized):
  - Nodes partitioned across 8 cores (graph parallel); per-core permutation
    sorts nodes by in-degree so adjacent 128-node tiles have similar max
    degree K.
  - Tiles are grouped into blocks of m tiles padded to a common slot count
    Kb; each block's whole neighbor gather is ONE batched indirect DMA
    (offset AP [128, m*Kb], one descriptor per edge slot).
  - Per layer: node phase projects features + attention dots with one
    matmul per 128-node tile against combined [WA|W|WD] (grouped 3-4 tiles
    per PSUM bank); the [als|h] table is AllGathered so every core can
    gather any source row.
  - Edge phase per block: softmax (no max-subtraction; logits bounded),
    alpha-weighting in place, then slot-axis segment reduction via ONE
    strided-view vector tensor_reduce (slot axis made innermost by AP
    permutation) — no per-slot matmuls.
  - Matmul inputs (x, h, weights) are bf16 (f32 PSUM accumulate); tables
    and softmax math stay f32; final output is f16 (cast to f32 on host).
  - Padding slots gather a dummy row (als=-100 -> exp ~ 0, h=0).
"""
import sys
import threading

sys.path.insert(0, "/opt/trn_rl_repo")

import numpy as np
import ml_dtypes

import jax
import jax.numpy as jnp
from jax.sharding import Mesh, PartitionSpec, NamedSharding
from jax.experimental.shard_map import shard_map

import concourse.bacc as bacc
import concourse.bass2jax as b2j
import concourse.tile as tile
from concourse import mybir
from concourse.bass import IndirectOffsetOnAxis, ds
from concourse.bass import ts as bass_ts
from concourse.masks import make_identity

# Warm the one-time cffi/pycparser ISA tables at import (~0.9 s) so the
# first Bacc build inside kernel() doesn't pay for it.
try:
    bacc.Bacc("TRN2", target_bir_lowering=False, debug=False,
              num_devices=1).isa
except Exception:
    pass

AF = mybir.ActivationFunctionType
ALU = mybir.AluOpType
AX = mybir.AxisListType

P = 128
NCORES = 8
LRELU = 0.2
LN_EPS = 1e-5

# problem dims (hardcoded per contract)
N_FULL = 100000
D_IN = 128
D_OUT = 64

SLOTS = 96     # max padded slots per block (m * Kb)
MBLK = 12      # max tiles per edge block
GB = 16        # gather-loop batch: columns fetched per For_i iteration

F16 = np.float16


# --------------------------------------------------------------------------
# host-side graph layout
# --------------------------------------------------------------------------

def prepare_layout(edge_index: np.ndarray, n: int):
    npc = n // NCORES
    nloc = ((npc + 1 + P - 1) // P) * P       # >=1 pad row per core
    nt = nloc // P
    nrows = NCORES * nloc

    loops = np.arange(n, dtype=np.int32)
    src = np.concatenate([loops, edge_index[0].astype(np.int32)])
    dst = np.concatenate([loops, edge_index[1].astype(np.int32)])

    deg = np.bincount(dst, minlength=n).astype(np.int32)  # incl self-loop

    dg = deg.reshape(NCORES, npc)
    order = np.argsort(dg, axis=1, kind="stable")                 # [8, npc]
    olds_sorted = order + (np.arange(NCORES) * npc)[:, None]      # old ids
    new_id = np.empty(n, np.int32)
    new_mat = np.arange(npc, dtype=np.int32)[None, :] + \
        (np.arange(NCORES, dtype=np.int32) * nloc)[:, None]
    new_id[olds_sorted.ravel()] = new_mat.ravel()

    nsrc = new_id[src]
    ndst = new_id[dst]

    degn = np.zeros(nrows, np.int32)
    degn[new_id] = deg
    K = degn.reshape(NCORES, nt, P).max(axis=(0, 2))
    K = np.maximum(K, 1).astype(np.int64)
    slots = max(SLOTS, int(K.max()))

    # greedy blocks of adjacent tiles padded to the block max degree
    blocks = []            # (t0, m, Kb, boff)
    colof = np.zeros(nt, np.int32)
    boff = 0
    t0 = 0
    while t0 < nt:
        m = 1
        Kb = int(K[t0])
        while (t0 + m < nt and m < MBLK
               and (m + 1) * max(Kb, int(K[t0 + m])) <= slots):
            Kb = max(Kb, int(K[t0 + m]))
            m += 1
        for j in range(m):
            colof[t0 + j] = boff + j * Kb
        blocks.append((t0, m, Kb, boff))
        boff += m * Kb
        t0 += m
    SUMK = ((boff + GB - 1) // GB) * GB   # pad so the gather loop tiles evenly

    idx = np.empty((NCORES, P, SUMK), dtype=np.int32)
    dummy = (np.arange(NCORES) * nloc + nloc - 1).astype(np.int32)
    idx[:] = dummy[:, None, None]

    order2 = np.argsort(ndst, kind="stable")
    s2 = nsrc[order2]
    d2 = ndst[order2]
    run_start = np.searchsorted(d2, np.arange(nrows, dtype=np.int32)).astype(np.int32)
    slot = np.arange(len(d2), dtype=np.int32) - run_start[d2]
    c_arr = d2 // nloc
    rank = d2 % nloc
    cols = colof[rank // P] + slot
    idx[c_arr, rank % P, cols] = s2

    return {
        "n": n, "npc": npc, "nloc": nloc, "nt": nt, "nrows": nrows,
        "olds_sorted": olds_sorted, "blocks": blocks, "SUMK": SUMK,
        "idx": idx, "K": K,
    }


# --------------------------------------------------------------------------
# device program
# --------------------------------------------------------------------------

class LayerSpec:
    def __init__(self, heads, ch, last, use_bias, use_gamma, use_beta):
        self.heads = heads
        self.ch = ch
        self.dh = heads * ch
        self.row = heads + self.dh         # [als(H) | h(dh)]
        self.ncols = self.row + heads      # + ald(H)
        self.last = last
        self.use_bias = use_bias
        self.use_gamma = use_gamma
        self.use_beta = use_beta


def build_nc(layout, specs):
    nloc, nt, nrows = layout["nloc"], layout["nt"], layout["nrows"]
    blocks, SUMK = layout["blocks"], layout["SUMK"]
    f32 = mybir.dt.float32
    f16 = mybir.dt.float16

    nc = bacc.Bacc("TRN2", target_bir_lowering=False, debug=False,
                   num_devices=NCORES)

    # ---- external I/O ----
    xT_d = nc.dram_tensor("xT", [P, nloc], f16, kind="ExternalInput")
    idx_d = nc.dram_tensor("idx", [P, SUMK], mybir.dt.int32, kind="ExternalInput")
    wall_d = [nc.dram_tensor(f"wall{i}", [P, s.ncols], f16, kind="ExternalInput")
              for i, s in enumerate(specs)]
    auxw = 32 + (3 * P * len(specs)
                 if any(s.use_bias or s.use_gamma or s.use_beta for s in specs)
                 else 0)
    aux_d = nc.dram_tensor("aux", [P, auxw], f32, kind="ExternalInput")
    # aux cols: [0:8]=-100 dummy als, [8]=LN eps, 32+li*384: [bias|gamma|beta]
    out_d = nc.dram_tensor("out", [nloc, specs[-1].dh], f16, kind="ExternalOutput")

    with tile.TileContext(nc) as tc:
        import contextlib
        ctx = contextlib.ExitStack()
        with ctx:
            cpool = ctx.enter_context(tc.tile_pool(name="const", bufs=1))
            dram = ctx.enter_context(tc.tile_pool(name="dram", bufs=1, space="DRAM"))
            npsum = ctx.enter_context(tc.tile_pool(name="npsum", bufs=2, space="PSUM"))
            tpsum = ctx.enter_context(tc.tile_pool(name="tpsum", bufs=2, space="PSUM"))
            gpool = ctx.enter_context(tc.tile_pool(name="gpool", bufs=2))
            work = ctx.enter_context(tc.tile_pool(name="work", bufs=2))
            spool = ctx.enter_context(tc.tile_pool(name="small", bufs=2))

            # ---- persistent SBUF ----
            hin = cpool.tile([P, nloc], f16)
            nc.sync.dma_start(hin[:], xT_d[:])
            idx_sb = cpool.tile([P, SUMK], mybir.dt.int32)
            nc.sync.dma_start(idx_sb[:], idx_d[:])
            aux = cpool.tile([P, auxw], f32)
            nc.sync.dma_start(aux[:], aux_d[:])
            identb = cpool.tile([P, P], f16)
            make_identity(nc, identb[:])
            ald_sb = cpool.tile([P, nt * 8], f32)
            ald2_sb = cpool.tile([P, nt], f32)

            walls = []
            for i, s in enumerate(specs):
                w = cpool.tile([P, s.ncols], f16, name=f"wall{i}_sb")
                nc.sync.dma_start(w[:], wall_d[i][:])
                walls.append(w)

            # per-layer DRAM tables
            tls = [dram.tile([nloc, s.row], f32, name=f"tl{i}")
                   for i, s in enumerate(specs)]
            tfs = [dram.tile([nrows, s.row], f32, name=f"tf{i}", addr_space="Shared")
                   for i, s in enumerate(specs)]

            # gather staging (data indirection: the indirect DMA's offset AP
            # stays static; a per-iteration copy feeds it fresh indices)
            gidx = cpool.tile([P, GB], mybir.dt.int32)
            grows = cpool.tile([P, GB, 136], f32)

            for li, s in enumerate(specs):
                wall = walls[li]
                H, ch, dh, row = s.heads, s.ch, s.dh, s.row
                tl, tf = tls[li], tfs[li]
                ald = ald_sb if H == 8 else ald2_sb

                # ---------- node phase (For_i over groups of gsz tiles) ----
                # matmul lhsT must be a static AP (walrus ldweights), so each
                # iteration stages the hin column block first.
                gsz = 7 if s.ncols * 7 <= 512 else 2        # nt = 98 = 2*7*7
                hstage = cpool.tile([P, gsz * P], f16, tag=f"hstage{li}",
                                    name=f"hstage{li}")
                pn = npsum.tile([P, gsz, s.ncols], f32, tag="pn")
                stage = work.tile([P, gsz, row], f32, tag="stage")
                with tc.For_i(0, nt // gsz) as gi:
                    nc.scalar.copy(hstage[:],
                                   hin[:, bass_ts(gi, gsz * P)])
                    for j in range(gsz):
                        nc.tensor.matmul(out=pn[:, j, :],
                                         lhsT=hstage[:, j * P:(j + 1) * P],
                                         rhs=wall[:], start=True, stop=True)
                    nc.scalar.copy(stage[:], pn[:, :, 0:row])
                    nc.scalar.copy(
                        ald[:, bass_ts(gi, gsz * H)].rearrange(
                            "p (m h) -> p m h", m=gsz),
                        pn[:, :, row:row + H])
                    nc.sync.dma_start(
                        tl[bass_ts(gi, gsz * P), :].rearrange(
                            "(j p) r -> p j r", p=P),
                        stage[:])

                # dummy row: overwrite als cols of last row with -100
                nc.sync.dma_start(tl[nloc - 1:nloc, 0:H],
                                  aux[0:1, 0:H])

                # ---------- allgather ----------
                # drain in-flight SWDGE DMAs: a collective triggered with
                # indirect-DMA descriptors in flight crashes the exec unit
                nc.gpsimd.dma_reset()
                nc.gpsimd.collective_compute(
                    "AllGather", ALU.bypass,
                    ins=[tl[:]], outs=[tf[:]],
                    replica_groups=[list(range(NCORES))],
                )

                # ---------- gather loop: stream all edge rows to DRAM ----
                gedge = dram.tile([P, SUMK, row], f32, tag="gedge",
                                  name=f"gedge{li}")
                with tc.For_i(0, SUMK, GB) as it:
                    nc.vector.tensor_copy(gidx[:], idx_sb[:, ds(it, GB)])
                    for b_ in range(GB):
                        nc.gpsimd.indirect_dma_start(
                            out=grows[:, b_, 0:row], out_offset=None,
                            in_=tf[:],
                            in_offset=IndirectOffsetOnAxis(
                                ap=gidx[:, b_:b_ + 1], axis=0),
                        )
                    nc.sync.dma_start(gedge[:, ds(it, GB), :],
                                      grows[:, :, 0:row])

                # ---------- edge phase (per block) ----------
                for (t0, m, Kb, boff) in blocks:
                    S = m * Kb
                    g = gpool.tile([P, S, row], f32, tag="g")
                    nc.sync.dma_start(g[:], gedge[:, boff:boff + S, :])
                    # logits l = als + ald  ([P, m, Kb, H] views)
                    lsb = work.tile([P, S, H], f32, tag="lsb")
                    nc.vector.tensor_tensor(
                        lsb[:].rearrange("p (m k) h -> p m k h", m=m),
                        g[:, :, 0:H].rearrange("p (m k) h -> p m k h", m=m),
                        ald[:, t0 * H:(t0 + m) * H].rearrange(
                            "p (m h) -> p m h", m=m)[:, :, None, :]
                        .to_broadcast([P, m, Kb, H]),
                        ALU.add)
                    # leaky relu: (l * 0.2) max l ; then ee = exp(l)
                    nc.vector.scalar_tensor_tensor(
                        lsb[:], lsb[:], LRELU, lsb[:], op0=ALU.mult, op1=ALU.max)
                    nc.scalar.activation(lsb[:], lsb[:], AF.Exp)
                    # msg h *= ee (per head)
                    gh = g[:, :, H:row].rearrange("p s (h c) -> p s h c", h=H)
                    nc.vector.tensor_tensor(
                        gh, gh,
                        lsb[:, :, :, None].to_broadcast([P, S, H, ch]),
                        ALU.mult)
                    # denominators: reduce ee over slot axis (innermost view)
                    den = spool.tile([P, m, H], f32, tag="den")
                    nc.vector.tensor_reduce(
                        den[:],
                        lsb[:].rearrange("p (m k) h -> p m h k", m=m),
                        axis=AX.X, op=ALU.add)
                    # messages: reduce weighted h over slot axis
                    msg = work.tile([P, m, dh], f32, tag="msg")
                    nc.vector.tensor_reduce(
                        msg[:],
                        g[:, :, H:row].rearrange("p (m k) r -> p m r k", m=m),
                        axis=AX.X, op=ALU.add)
                    # normalize by denominator
                    rec = spool.tile([P, m, H], f32, tag="rec")
                    nc.vector.reciprocal(rec[:], den[:])
                    msg4 = msg[:].rearrange("p m (h c) -> p m h c", h=H)
                    nc.vector.tensor_tensor(
                        msg4, msg4,
                        rec[:, :, :, None].to_broadcast([P, m, H, ch]),
                        ALU.mult)
                    if s.use_bias:
                        nc.vector.tensor_tensor(
                            msg[:], msg[:],
                            aux[:, None, 32 + li * 3 * P:32 + li * 3 * P + dh]
                            .to_broadcast([P, m, dh]),
                            ALU.add)

                    if not s.last:
                        # ---- layer norm + relu (per block, vector ops) ----
                        s1 = spool.tile([P, m], f32, tag="s1")
                        nc.vector.tensor_reduce(s1[:], msg[:], axis=AX.X,
                                                op=ALU.add)
                        sq = work.tile([P, m, dh], f32, tag="sq")
                        nc.scalar.activation(sq[:], msg[:], AF.Square)
                        s2 = spool.tile([P, m], f32, tag="s2")
                        nc.vector.tensor_reduce(s2[:], sq[:], axis=AX.X,
                                                op=ALU.add)
                        mu = spool.tile([P, m], f32, tag="mu")
                        nc.vector.tensor_scalar_mul(mu[:], s1[:], 1.0 / dh)
                        ex2 = spool.tile([P, m], f32, tag="ex2")
                        nc.vector.tensor_scalar_mul(ex2[:], s2[:], 1.0 / dh)
                        mu2 = spool.tile([P, m], f32, tag="mu2")
                        nc.vector.tensor_tensor(mu2[:], mu[:], mu[:], ALU.mult)
                        var = spool.tile([P, m], f32, tag="var")
                        nc.vector.tensor_tensor(var[:], ex2[:], mu2[:],
                                                ALU.subtract)
                        sd = spool.tile([P, m], f32, tag="sd")
                        nc.scalar.activation(sd[:], var[:], AF.Sqrt,
                                             bias=aux[:, 8:9])
                        rstd = spool.tile([P, m], f32, tag="rstd")
                        nc.vector.reciprocal(rstd[:], sd[:])
                        # xn = (msg - mu) * rstd  (reuse sq buffer)
                        nc.vector.tensor_tensor(
                            sq[:], msg[:],
                            mu[:, :, None].to_broadcast([P, m, dh]),
                            ALU.subtract)
                        nc.vector.tensor_tensor(
                            sq[:], sq[:],
                            rstd[:, :, None].to_broadcast([P, m, dh]),
                            ALU.mult)
                        if s.use_gamma:
                            nc.vector.tensor_tensor(
                                sq[:], sq[:],
                                aux[:, None, 32 + li * 3 * P + P:
                                    32 + li * 3 * P + P + dh]
                                .to_broadcast([P, m, dh]), ALU.mult)
                        if s.use_beta:
                            nc.vector.tensor_tensor(
                                sq[:], sq[:],
                                aux[:, None, 32 + li * 3 * P + 2 * P:
                                    32 + li * 3 * P + 2 * P + dh]
                                .to_broadcast([P, m, dh]), ALU.add)
                        hn = work.tile([P, m, dh], f16, tag="hn")
                        nc.vector.tensor_scalar_max(hn[:], sq[:], 0.0)
                        # transpose each tile back into hin (feature-major)
                        for j in range(m):
                            pt = tpsum.tile([P, P], f16, tag="pt")
                            nc.tensor.transpose(pt[:], hn[:, j, :], identb[:])
                            nc.scalar.copy(
                                hin[:, (t0 + j) * P:(t0 + j + 1) * P], pt[:])
                    else:
                        # ---- log_softmax + output DMA ----
                        mxn = spool.tile([P, m], f32, tag="mxn")
                        nc.vector.tensor_reduce(mxn[:], msg[:], axis=AX.X,
                                                op=ALU.max, negate=True)
                        tsb = work.tile([P, m, dh], f32, tag="tsb")
                        nc.vector.tensor_tensor(
                            tsb[:], msg[:],
                            mxn[:, :, None].to_broadcast([P, m, dh]),
                            ALU.add)
                        nc.scalar.activation(msg[:], tsb[:], AF.Exp)
                        ssum = spool.tile([P, m], f32, tag="ssum")
                        nc.vector.tensor_reduce(ssum[:], msg[:], axis=AX.X,
                                                op=ALU.add)
                        lns = spool.tile([P, m], f32, tag="lns")
                        nc.scalar.activation(lns[:], ssum[:], AF.Ln)
                        of = work.tile([P, m, dh], f16, tag="of")
                        nc.vector.tensor_tensor(
                            of[:], tsb[:],
                            lns[:, :, None].to_broadcast([P, m, dh]),
                            ALU.subtract)
                        nc.sync.dma_start(
                            out_d[t0 * P:(t0 + m) * P, :].rearrange(
                                "(j p) c -> p j c", p=P),
                            of[:])

    nc.compile()
    return nc


# --------------------------------------------------------------------------
# host wrapper
# --------------------------------------------------------------------------

def _block_diag_a(a, heads, ch):
    """[heads*ch, heads]: col h nonzero only on head h's channels."""
    out = np.zeros((heads * ch, heads), dtype=np.float32)
    for h in range(heads):
        out[h * ch:(h + 1) * ch, h] = a[h]
    return out


def _device_mesh():
    devices = jax.devices()[:NCORES]
    mesh = Mesh(np.asarray(devices), ("core",))
    return mesh, NamedSharding(mesh, PartitionSpec("core"))


def _run_pjrt(nc, mesh, sh, dev_in):
    """Execute the prebuilt Bass module via PJRT against inputs that are
    already resident on the devices (adapted from bass2jax.run_bass_via_pjrt,
    minus host-side zero-output transfers)."""
    b2j.install_neuronx_cc_hook()
    partition_name = nc.partition_id_tensor.name if nc.partition_id_tensor else None
    in_names, out_names, out_avals = [], [], []
    for alloc in nc.m.functions[0].allocations:
        if not isinstance(alloc, mybir.MemoryLocationSet):
            continue
        name = alloc.memorylocations[0].name
        if alloc.kind == "ExternalInput":
            if name != partition_name:
                in_names.append(name)
        elif alloc.kind == "ExternalOutput":
            out_names.append(name)
            out_avals.append(jax.core.ShapedArray(
                tuple(alloc.tensor_shape), mybir.dt.np(alloc.dtype)))
    n_params = len(in_names)
    n_outs = len(out_avals)
    all_names = list(in_names) + out_names
    if partition_name is not None:
        all_names.append(partition_name)
    donate = tuple(range(n_params, n_params + n_outs))

    def _body(*args):
        operands = list(args)
        if partition_name is not None:
            operands.append(b2j.partition_id_tensor())
        outs = b2j._bass_exec_p.bind(
            *operands, out_avals=tuple(out_avals), in_names=tuple(all_names),
            out_names=tuple(out_names), lowering_input_output_aliases=(),
            sim_require_finite=True, sim_require_nnan=True, nc=nc)
        return tuple(outs)

    in_specs = (PartitionSpec("core"),) * (n_params + n_outs)
    out_specs = (PartitionSpec("core"),) * n_outs
    fn = jax.jit(shard_map(_body, mesh=mesh, in_specs=in_specs,
                           out_specs=out_specs, check_rep=False),
                 donate_argnums=donate, keep_unused=True)
    # donated output buffers were pre-transferred (see run_gat "__zero_<name>")
    zeros = [dev_in[f"__zero_{nm}"] for nm in out_names]
    args = [dev_in[nm] for nm in in_names] + zeros
    outs = fn(*args)
    return {name: np.asarray(outs[i]).reshape(NCORES, *out_avals[i].shape)
            for i, name in enumerate(out_names)}


def run_gat(inputs, n=N_FULL):
    x = np.asarray(inputs["x"], dtype=np.float32)
    edge_index = np.asarray(inputs["edge_index"])
    lay = prepare_layout(edge_index, n)
    nloc, npc = lay["nloc"], lay["npc"]

    W = [np.asarray(inputs[f"W{i}"], dtype=np.float32) for i in range(3)]
    a_s = [np.asarray(inputs[f"as{i}"], dtype=np.float32) for i in range(3)]
    a_d = [np.asarray(inputs[f"ad{i}"], dtype=np.float32) for i in range(3)]
    b = [np.asarray(inputs[f"b{i}"], dtype=np.float32) for i in range(3)]
    ln_g = [np.asarray(inputs["ln1_g"], np.float32),
            np.asarray(inputs["ln2_g"], np.float32)]
    ln_b = [np.asarray(inputs["ln1_b"], np.float32),
            np.asarray(inputs["ln2_b"], np.float32)]

    hc = [(8, 16), (8, 16), (1, 64)]
    specs = []
    for i, (heads, ch) in enumerate(hc):
        use_bias = bool(np.any(b[i] != 0.0))
        use_g = i < 2 and bool(np.any(ln_g[i] != 1.0))
        use_b = i < 2 and bool(np.any(ln_b[i] != 0.0))
        specs.append(LayerSpec(heads, ch, i == 2, use_bias, use_g, use_b))

    # per-layer combined weights [WA(H) | W(dh) | WD(H)], bf16
    wall_np = []
    for i, s in enumerate(specs):
        din = W[i].shape[0]
        bd_s = _block_diag_a(a_s[i].reshape(s.heads, s.ch), s.heads, s.ch)
        bd_d = _block_diag_a(a_d[i].reshape(s.heads, s.ch), s.heads, s.ch)
        m = np.zeros((P, s.ncols), dtype=np.float32)
        m[:din, 0:s.heads] = W[i] @ bd_s
        m[:din, s.heads:s.heads + s.dh] = W[i]
        m[:din, s.heads + s.dh:] = W[i] @ bd_d
        wall_np.append(m.astype(F16))

    wide = any(s.use_bias or s.use_gamma or s.use_beta for s in specs)
    auxw = 32 + (3 * P * len(specs) if wide else 0)
    aux_np = np.zeros((P, auxw), dtype=np.float32)
    aux_np[:, 0:8] = -100.0
    aux_np[:, 8] = LN_EPS
    if wide:
        for i, s in enumerate(specs):
            aux_np[:, 32 + i * 3 * P:32 + i * 3 * P + s.dh] = b[i][None, :]
            if i < 2:
                aux_np[:, 32 + i * 3 * P + P:32 + i * 3 * P + P + s.dh] = \
                    ln_g[i][None, :]
                aux_np[:, 32 + i * 3 * P + 2 * P:32 + i * 3 * P + 2 * P + s.dh] = \
                    ln_b[i][None, :]

    # per-core transposed bf16 features
    xg = x[lay["olds_sorted"]]          # [8, npc, 128]
    xT_all = np.zeros((NCORES, P, nloc), dtype=F16)
    xT_all[:, :, :npc] = xg.transpose(0, 2, 1)

    # kick off host->device transfers NOW; they stream over the (slow) axon
    # tunnel while we trace + compile the kernel below.
    mesh, sh = _device_mesh()
    rep = lambda a: np.broadcast_to(a, (NCORES,) + a.shape).reshape(
        NCORES * a.shape[0], *a.shape[1:])
    concat = {
        "xT": xT_all.reshape(NCORES * P, nloc),
        "idx": lay["idx"].reshape(NCORES * P, lay["SUMK"]),
        "aux": rep(aux_np),
    }
    for i in range(3):
        concat[f"wall{i}"] = rep(wall_np[i])
    concat["__zero_out"] = np.zeros((NCORES * nloc, specs[-1].dh),
                                    dtype=np.float16)
    names = list(concat.keys())
    dev_in = {}
    def _put():
        put = jax.device_put([concat[k] for k in names], sh)
        dev_in.update(zip(names, put))
    put_thread = threading.Thread(target=_put)
    put_thread.start()

    nc = build_nc(lay, specs)
    put_thread.join()
    if nc.dbg_addr is not None:
        dev_in[nc.dbg_addr.name] = jax.device_put(
            np.zeros((NCORES, 2), np.uint32), sh)

    res = _run_pjrt(nc, mesh, sh, dev_in)

    out = res["out"][:, :npc, :].astype(np.float32)   # [8, npc, 64]
    full = np.empty((n, specs[-1].dh), dtype=np.float32)
    full[lay["olds_sorted"].ravel()] = out.reshape(NCORES * npc, -1)
    return full


def kernel(**inputs) -> np.ndarray:
    return run_gat(inputs, n=N_FULL)
